# revision 33
# baseline (speedup 1.0000x reference)
"""Trainium2 Bass kernel for nn_PraxisAttention (causal linear attention).

Sharding: 8 cores = 4 batches x 2 head-groups (tensor-parallel over the 16
heads, per the sharding hint). Core c handles batch c//2 and heads
[8*(c%2), 8*(c%2)+8). Each core computes q/k/v projections for its 1024
feature columns (bf16 matmuls, fp32 accumulate), the elu(x)+1 feature map
(min(exp(x),1)+relu(x)), causal cumulative sums over the full 4096-token
sequence via DVE prefix scans, z = per-head dot(q, k_cum), and the
row-sharded output projection, which yields partial sums. The host adds the
two partials per batch, re-adds bo, and transposes back.

The z reduction runs on the GPSIMD (Pool) engine via partition_all_reduce,
which also broadcasts the per-token sum to all 128 partitions, so neither
the one-hot z-reduce matmuls nor the 1/z broadcast matmuls of the earlier
revision touch the PE: the PE executes projection/output GEMM streaming
only. 1/z is a DVE reciprocal; the (reciprocal, w=q*kvcum/z) pair for head
h is deferred until after head h+1's projection is issued so the in-order
DVE queue never waits on the Pool engine's latency. The reference's +EPS
(1e-6) on z is dropped: z is a sum of 128 strictly positive products (~1e2),
so the term is ~1e-8 relative, far below the bf16 noise floor.

The chunk loop is software-pipelined: chunk c's output projection is
deferred until after chunk c+1's projections are issued on the PE, so the
PE never stalls waiting for the w tiles' Act/DVE/Pool chain.

The attention_mask input is all-ones per the problem spec (a zero entry
would make the reference divide by zero), so multiplying k/v/z by it is an
identity and is skipped on device.

Numerics: matmul operands bf16 (fp32 PSUM accumulation); all attention-core
intermediates fp32; final partials stored/summed fp32.
"""

import sys

sys.path.insert(0, "/opt/trn_rl_repo")

import numpy as np
import ml_dtypes

BF16 = ml_dtypes.bfloat16

# Problem constants
B, L, D = 4, 4096, 2048
H, DH = 16, 128
N_CORES = 8
HPC = 8        # heads per core
FPC = HPC * DH  # feature columns per core (1024)
CH = 512       # tokens per chunk
NCH = L // CH  # 8 chunks
KT = D // 128  # 16 k-tiles (projection contraction)
KO = FPC // 128  # 8 k-tiles (output projection contraction)
NT = D // 128  # 16 output feature tiles

_CACHE = {}


def _build_program(loop_r=None, pipelined=True):
    """Build the per-core program. loop_r (timing only): wrap the whole body
    in a hardware For_i loop executing it loop_r times per dispatch."""
    import concourse.tile as tile
    from concourse import mybir, bacc

    fp32 = mybir.dt.float32
    bf16 = mybir.dt.bfloat16

    nc = bacc.Bacc("TRN2", target_bir_lowering=False, debug=False,
                   enable_asserts=True, num_devices=N_CORES)

    # Inputs (host pre-arranged, see kernel()):
    # xk[c][p][kk*CH+t] = x[b].T[kk*128+p, c*CH+t]
    xk_d = nc.dram_tensor("xk", [NCH, 128, KT * CH], bf16, kind="ExternalInput").ap()
    # wX[h][p][kk*128+j] = W[kk*128+p, h*128+j] (column-sharded slice)
    wq_d = nc.dram_tensor("wq", [HPC, 128, KT * 128], bf16, kind="ExternalInput").ap()
    wk_d = nc.dram_tensor("wk", [HPC, 128, KT * 128], bf16, kind="ExternalInput").ap()
    wv_d = nc.dram_tensor("wv", [HPC, 128, KT * 128], bf16, kind="ExternalInput").ap()
    # wo[n][p][hh*128+j] = Wo[rows][hh*128+p, n*128+j] (row-sharded slice)
    wo_d = nc.dram_tensor("wo", [NT, 128, KO * 128], bf16, kind="ExternalInput").ap()
    # Output: partial yT[n][p][c*CH+t] = sum over this core's features
    y_d = nc.dram_tensor("yT", [NT, 128, L], fp32, kind="ExternalOutput").ap()

    with tile.TileContext(nc) as tc:
        with (
            tc.tile_pool(name="const", bufs=1) as constp,
            tc.tile_pool(name="carry", bufs=1) as carryp,
            tc.tile_pool(name="xk", bufs=2) as xkp,
            tc.tile_pool(name="wts", bufs=6) as wtsp,
            tc.tile_pool(name="wo", bufs=8) as wop,
            tc.tile_pool(name="tmp", bufs=8) as tmpp,
            tc.tile_pool(name="w1", bufs=3) as w1p,
            tc.tile_pool(name="zs", bufs=3) as zsp,
            tc.tile_pool(name="zinv", bufs=3) as zvp,
            tc.tile_pool(name="kcum", bufs=9) as kcump,
            tc.tile_pool(name="kvcum", bufs=9) as kvcump,
            tc.tile_pool(name="qf", bufs=9) as qfp,
            tc.tile_pool(name="wtile", bufs=17) as wtp,
            tc.tile_pool(name="outs", bufs=4) as outp,
            tc.tile_pool(name="pp", bufs=3, space="PSUM") as pp,
            tc.tile_pool(name="po", bufs=5, space="PSUM") as pop,
        ):
            # wk0 first in the DMA queue: HWDGE dispatch is ~0.6us per DMA
            # shared across queues, so one 0.5MB transfer beats pieces.
            wk0 = constp.tile([128, KT * 128], bf16)
            nc.sync.dma_start(wk0[:], wk_d[0])

            ck = carryp.tile([128, HPC], fp32)   # k-cumsum carries
            ckv = carryp.tile([128, HPC], fp32)  # kv-cumsum carries

            # PE p-state warmup: the PE runs at a reduced clock for the
            # first ~3us of a busy streak. Burn that ramp on dummy matmuls
            # DURING the startup DMA wait, so the real projections start at
            # full clock. The warm tile is a single partition (contraction
            # K=1 is legal and streams the same 512 columns), so its Pool
            # memset is ~2.5KB and the PE starts within ~0.5us of t=0.
            # Once per dispatch, outside the timing loop.
            warm = constp.tile([1, 512], bf16)
            nc.gpsimd.memset(warm[:], 0.0)
            pwarm = pp.tile([128, CH], fp32, tag="pp")
            for _ in range(7):
                nc.tensor.matmul(pwarm[:], warm[:, 0:128], warm[:],
                                 start=True, stop=True)

            import contextlib
            loop_ctx = (tc.For_i(0, loop_r, 1) if loop_r
                        else contextlib.nullcontext())
            with loop_ctx:
                _body(nc, tc, mybir, xk_d, wq_d, wk_d, wv_d, wo_d, y_d,
                      wk0, ck, ckv,
                      xkp, wtsp, wop, tmpp, w1p, zsp, zvp, kcump, kvcump,
                      qfp, wtp, outp, pp, pop, pipelined)

    nc.compile()
    return nc


def _body(nc, tc, mybir, xk_d, wq_d, wk_d, wv_d, wo_d, y_d,
          wk0, ck, ckv,
          xkp, wtsp, wop, tmpp, w1p, zsp, zvp, kcump, kvcum_p, qfp, wtp,
          outp, pp, pop, pipelined=True):
    """Software-pipelined chunk loop.

    PE issue order: A(0) B1(0) A(1) B1(1) ZO(0) A(2) B1(2) ZO(1) ... ZO(7).
    ZO(c) (the output projection) is deferred until after chunk c+1's
    projections, so the PE reaches it long after the w tiles' Act/DVE/Pool
    producer chain has drained.
    """
    from concourse import bass_isa
    fp32 = mybir.dt.float32
    bf16 = mybir.dt.bfloat16
    AL = mybir.AluOpType
    AF = mybir.ActivationFunctionType

    nc.vector.memset(ck[:], 0.0)
    nc.vector.memset(ckv[:], 0.0)

    def phase_a(c, xk, wk_pre=None):
        # k/v projections, feature map, causal scans
        kc_tiles, kvc_tiles = [], []
        for h in range(HPC):
            if h == 0 and wk_pre is not None:
                wkh = wk_pre
            else:
                wkh = wtsp.tile([128, KT * 128], bf16, tag="wts")
                nc.sync.dma_start(wkh[:], wk_d[h])
            pk = pp.tile([128, CH], fp32, tag="pp")
            for kk in range(KT):
                nc.tensor.matmul(
                    pk[:], wkh[:, kk * 128:(kk + 1) * 128],
                    xk[:, kk * CH:(kk + 1) * CH],
                    start=(kk == 0), stop=(kk == KT - 1))
            e = tmpp.tile([128, CH], fp32, tag="tmp")
            nc.scalar.activation(e[:], pk[:], AF.Exp)
            r = tmpp.tile([128, CH], fp32, tag="tmp")
            nc.scalar.activation(r[:], pk[:], AF.Relu)
            kf = tmpp.tile([128, CH], fp32, tag="tmp")
            nc.vector.scalar_tensor_tensor(
                kf[:], e[:], 1.0, r[:], AL.min, AL.add)

            wvh = wtsp.tile([128, KT * 128], bf16, tag="wts")
            nc.sync.dma_start(wvh[:], wv_d[h])
            pv = pp.tile([128, CH], fp32, tag="pp")
            for kk in range(KT):
                nc.tensor.matmul(
                    pv[:], wvh[:, kk * 128:(kk + 1) * 128],
                    xk[:, kk * CH:(kk + 1) * CH],
                    start=(kk == 0), stop=(kk == KT - 1))
            kv = tmpp.tile([128, CH], fp32, tag="tmp")
            # kv = kf * v, reading v straight from PSUM
            nc.vector.tensor_tensor(kv[:], kf[:], pv[:], AL.mult)

            kc = kcump.tile([128, CH], fp32, tag="kcum")
            init_k = 0.0 if c == 0 else ck[:, h:h + 1]
            nc.vector.tensor_tensor_scan(
                kc[:], kf[:], kf[:], init_k, AL.add, AL.bypass)
            nc.vector.tensor_copy(ck[:, h:h + 1], kc[:, CH - 1:CH])

            kvc = kvcum_p.tile([128, CH], fp32, tag="kvcum")
            init_kv = 0.0 if c == 0 else ckv[:, h:h + 1]
            nc.vector.tensor_tensor_scan(
                kvc[:], kv[:], kv[:], init_kv, AL.add, AL.bypass)
            nc.vector.tensor_copy(ckv[:, h:h + 1], kvc[:, CH - 1:CH])
            kc_tiles.append(kc)
            kvc_tiles.append(kvc)
        return kc_tiles, kvc_tiles

    def phase_b1(c, xk, kc_tiles, kvc_tiles):
        # q projection, feature map, z reduction on the Pool engine, and
        # w = qf * kvcum / z. The (reciprocal, w) pair for head h is
        # deferred until after head h+1's projection matmuls and DVE work
        # are issued, so the in-order DVE queue reaches the reciprocal well
        # after the Pool engine's partition_all_reduce has completed.
        w_tiles = []
        zpend = None  # (s_tile, w1_tile) awaiting reciprocal + final mult
        def flush(zpend):
            s, w1 = zpend
            zinv = zvp.tile([128, CH], fp32, tag="zinv")
            nc.vector.reciprocal(zinv[:], s[:])
            wh = wtp.tile([128, CH], bf16, tag="wtile")
            nc.vector.tensor_tensor(wh[:], w1[:], zinv[:], AL.mult)
            w_tiles.append(wh)
        for h in range(HPC):
            wqh = wtsp.tile([128, KT * 128], bf16, tag="wts")
            nc.sync.dma_start(wqh[:], wq_d[h])
            pq = pp.tile([128, CH], fp32, tag="pp")
            for kk in range(KT):
                nc.tensor.matmul(
                    pq[:], wqh[:, kk * 128:(kk + 1) * 128],
                    xk[:, kk * CH:(kk + 1) * CH],
                    start=(kk == 0), stop=(kk == KT - 1))
            eq = tmpp.tile([128, CH], fp32, tag="tmp")
            nc.scalar.activation(eq[:], pq[:], AF.Exp)
            rq = tmpp.tile([128, CH], fp32, tag="tmp")
            nc.scalar.activation(rq[:], pq[:], AF.Relu)
            qf = qfp.tile([128, CH], fp32, tag="qf")
            nc.vector.scalar_tensor_tensor(
                qf[:], eq[:], 1.0, rq[:], AL.min, AL.add)
            p = tmpp.tile([128, CH], fp32, tag="tmp")
            nc.vector.tensor_tensor(p[:], qf[:], kc_tiles[h][:], AL.mult)
            s = zsp.tile([128, CH], fp32, tag="zs")
            nc.gpsimd.partition_all_reduce(
                s[:], p[:], channels=128, reduce_op=bass_isa.ReduceOp.add)
            w1 = w1p.tile([128, CH], fp32, tag="w1")
            nc.vector.tensor_tensor(w1[:], qf[:], kvc_tiles[h][:], AL.mult)
            if zpend is not None:
                flush(zpend)
            zpend = (s, w1)
        flush(zpend)
        return w_tiles

    def phase_zo_out(c, w_tiles, last=False):
        # Row-sharded output projection for chunk c. First half: groups of
        # 4 output tiles with the head (contraction) loop OUTER, so on the
        # final chunk (whose ZO is not deferred behind another chunk's
        # projections) the PE's first matmuls consume only the early w
        # tiles while the last heads' Act/DVE/Pool chain drains. Second
        # half: one output tile at a time, so the PSUM->SBUF copies and
        # output DMAs spread out instead of bunching after the last
        # matmul. Output DMAs alternate between the SP and Activation
        # HWDGE queues.
        def emit_out(n, po):
            ot = outp.tile([128, CH], fp32)
            if n == NT - 1:
                # last tile of the chunk: split the PSUM drain + store
                # across both engines/queues to halve the end-of-chunk
                # serial tail after the final matmul
                hc = CH // 2
                nc.scalar.copy(ot[:, 0:hc], po[:, 0:hc])
                nc.vector.tensor_copy(ot[:, hc:CH], po[:, hc:CH])
                nc.scalar.dma_start(
                    y_d[n, :, c * CH:c * CH + hc], ot[:, 0:hc])
                nc.sync.dma_start(
                    y_d[n, :, c * CH + hc:(c + 1) * CH], ot[:, hc:CH])
                return
            nc.scalar.copy(ot[:], po[:])
            eng = nc.sync if n % 2 == 0 else nc.scalar
            eng.dma_start(y_d[n, :, c * CH:(c + 1) * CH], ot[:])
        # prefetch the first two groups' weight tiles (fills all 8 wo
        # bufs) so the first Ldweights of this ZO phase never waits on a
        # just-issued DMA
        wos8 = []
        for n in range(8):
            won = wop.tile([128, KO * 128], bf16, tag="wo")
            nc.sync.dma_start(won[:], wo_d[n])
            wos8.append(won)
        for g in (0, 4):
            wos, pos = [], []
            for n in range(g, g + 4):
                wos.append(wos8[n])
                po = pop.tile([128, CH], fp32)
                pos.append(po)
            for hh in range(KO):
                for i in range(4):
                    nc.tensor.matmul(
                        pos[i][:], wos[i][:, hh * 128:(hh + 1) * 128],
                        w_tiles[hh][:],
                        start=(hh == 0), stop=(hh == KO - 1))
            for i, n in enumerate(range(g, g + 4)):
                emit_out(n, pos[i])
        for batch in (range(8, 12), range(12, NT)):
            wons = []
            for n in batch:
                won = wop.tile([128, KO * 128], bf16, tag="wo")
                nc.sync.dma_start(won[:], wo_d[n])
                wons.append(won)
            for n, won in zip(batch, wons):
                po = pop.tile([128, CH], fp32)
                if last and n == NT - 1:
                    # very last tile of the program: accumulate it as two
                    # half-width PSUM groups so the end-of-program drain
                    # follows a half-size copy + DMA instead of a full one
                    hc = CH // 2
                    po2 = pop.tile([128, CH], fp32, tag="po")
                    for part, (lo, hi, pb) in enumerate(
                            ((0, hc, po), (hc, CH, po2))):
                        for hh in range(KO):
                            nc.tensor.matmul(
                                pb[:, 0:hc],
                                won[:, hh * 128:(hh + 1) * 128],
                                w_tiles[hh][:, lo:hi],
                                start=(hh == 0), stop=(hh == KO - 1))
                        ot = outp.tile([128, hc], fp32)
                        if part == 0:
                            nc.scalar.copy(ot[:], pb[:, 0:hc])
                            nc.scalar.dma_start(
                                y_d[n, :, c * CH + lo:c * CH + hi], ot[:])
                        else:
                            nc.vector.tensor_copy(ot[:], pb[:, 0:hc])
                            nc.sync.dma_start(
                                y_d[n, :, c * CH + lo:c * CH + hi], ot[:])
                    continue
                for hh in range(KO):
                    nc.tensor.matmul(
                        po[:], won[:, hh * 128:(hh + 1) * 128],
                        w_tiles[hh][:],
                        start=(hh == 0), stop=(hh == KO - 1))
                emit_out(n, po)

    pend = None  # (c, w_tiles) awaiting the deferred output projection
    for c in range(NCH):
        xk = xkp.tile([128, KT * CH], bf16)
        if c == 0:
            # startup: DMA transfers serialize globally in DISPATCH order
            # (one ~360 GB/s pipe; the two HWDGE queues only parallelize
            # the ~0.6us/DMA dispatch). Alternating chunk 0's x pieces
            # across the queues makes dispatch order equal consumption
            # order, so delivery tracks the PE's streaming.
            nc.scalar.dma_start(xk[:, 0:CH], xk_d[c][:, 0:CH])
            nc.sync.dma_start(xk[:, CH:4 * CH], xk_d[c][:, CH:4 * CH])
            nc.scalar.dma_start(xk[:, 4 * CH:7 * CH],
                                xk_d[c][:, 4 * CH:7 * CH])
            nc.sync.dma_start(xk[:, 7 * CH:10 * CH],
                              xk_d[c][:, 7 * CH:10 * CH])
            nc.scalar.dma_start(xk[:, 10 * CH:13 * CH],
                                xk_d[c][:, 10 * CH:13 * CH])
            nc.sync.dma_start(xk[:, 13 * CH:KT * CH],
                              xk_d[c][:, 13 * CH:KT * CH])
        else:
            nc.sync.dma_start(xk[:], xk_d[c])
        kc_tiles, kvc_tiles = phase_a(c, xk, wk0 if c == 0 else None)
        w_tiles = phase_b1(c, xk, kc_tiles, kvc_tiles)
        if not pipelined:
            phase_zo_out(c, w_tiles)
            continue
        if pend is not None:
            phase_zo_out(*pend)
        pend = (c, w_tiles)
    if pend is not None:
        phase_zo_out(*pend, last=True)


def _get_program():
    if "nc" not in _CACHE:
        _CACHE["nc"] = _build_program()
    return _CACHE["nc"]


def _prep_inputs(x, Wq, Wk, Wv, Wo):
    """Host-side shard + rearrange + cast. Returns per-core input maps."""
    def arrange_w_cols(W, g):
        # W[:, g*FPC:(g+1)*FPC] -> [HPC, 128, KT*128]
        Ws = np.ascontiguousarray(W[:, g * FPC:(g + 1) * FPC]).astype(BF16)
        return np.ascontiguousarray(
            Ws.reshape(KT, 128, HPC, 128).transpose(2, 1, 0, 3)
        ).reshape(HPC, 128, KT * 128)

    def arrange_wo_rows(W, g):
        # W[g*FPC:(g+1)*FPC, :] -> [NT, 128, KO*128]
        Ws = np.ascontiguousarray(W[g * FPC:(g + 1) * FPC, :]).astype(BF16)
        return np.ascontiguousarray(
            Ws.reshape(KO, 128, NT, 128).transpose(2, 1, 0, 3)
        ).reshape(NT, 128, KO * 128)

    w_by_g = []
    for g in range(2):
        w_by_g.append({
            "wq": arrange_w_cols(Wq, g),
            "wk": arrange_w_cols(Wk, g),
            "wv": arrange_w_cols(Wv, g),
            "wo": arrange_wo_rows(Wo, g),
        })

    xk_by_b = []
    for b in range(B):
        xT = np.ascontiguousarray(x[b].T).astype(BF16)  # [D, L]
        xk = np.ascontiguousarray(
            xT.reshape(KT, 128, NCH, CH).transpose(2, 1, 0, 3)
        ).reshape(NCH, 128, KT * CH)
        xk_by_b.append(xk)

    in_maps = []
    for c in range(N_CORES):
        b, g = c // 2, c % 2
        m = {"xk": xk_by_b[b]}
        m.update(w_by_g[g])
        in_maps.append(m)
    return in_maps


def _gather_output(results, bo):
    out = np.empty((B, L, D), np.float32)
    for b in range(B):
        yp = results[2 * b]["yT"] + results[2 * b + 1]["yT"]  # [NT,128,L]
        # yT[n, p, t] = out[t, n*128+p]
        out[b] = yp.reshape(NT * 128, L).T + bo[None, :]
    return out


def kernel(x, attention_mask, Wq, bq, Wk, bk, Wv, bv, Wo, bo, **_ignored):
    from concourse.bass_utils import run_bass_kernel_spmd

    x = np.asarray(x, np.float32)
    nc = _get_program()
    # bq/bk/bv are zero in this problem; q/k/v biases are additive constants
    # folded on host would be wrong (nonlinear feature map), so assert.
    assert not np.any(bq) and not np.any(bk) and not np.any(bv), \
        "kernel compiled for zero q/k/v biases"
    in_maps = _prep_inputs(x, np.asarray(Wq), np.asarray(Wk), np.asarray(Wv),
                           np.asarray(Wo))
    res = run_bass_kernel_spmd(nc, in_maps, list(range(N_CORES)))
    return _gather_output(res.results, np.asarray(bo, np.float32))
